# revision 13
# baseline (speedup 1.0000x reference)
"""PolymerDistance loss kernel for 8 Trainium2 NeuronCores.

Math (per molecule m of exactly 1024 atoms, Kabsch-style loss):
  loss[m] = var1 + var2 - 2*mean(sigma)  where sigma are singular values of the
  3x3 cross covariance of centered coords, with the smallest one sign-corrected
  by sign(det(cov)).

Device strategy:
  - Shard 4096 molecules evenly: 512 molecules (524288 atoms) per core. No
    collectives needed (molecules never straddle a shard).
  - Host pre-pass converts coords to component-separated fp16 ([3, N] per
    tensor). Input rounding error washes out over the 1024-atom averages
    (measured: identical L2 error to an all-f32 device path), while halving
    HBM traffic and enabling the DVE 2x perf mode (16-bit, unit-stride).
    All accumulation stays fp32 on device.
  - Per core, 4 groups of 128 molecules; one molecule per SBUF partition,
    free dim = [3 components x 1024 atoms] contiguous per component.
  - Sufficient statistics per molecule (18 f32 sums): 9 crosses
    sum(x2_i*x1_j), 6 per-component square sums, 3+3 first moments.
    Fused multiply+reduce via scalar_tensor_tensor(accum_out) on VectorE
    (moments multiply against a constant ones tile); ScalarE takes the
    squares (activation Square + accum_out) plus one moment to balance.
  - Tiny [M] stage stays on device: sigma1+sigma2+sign(det)*sigma3 is the
    largest root of  s^4 - 2*e1*s^2 - 8*det*s + (e1^2 - 4*e2)  where
    e1 = tr(C^T C) = |C|_F^2, e2 = (e1^2 - |C^T C|_F^2)/2, for C = n*cov.
    Solved with a tight initializer + 3 Newton steps, all elementwise on
    [128, 4] tiles. No trig, single Sqrt activation table.
"""

import sys

import numpy as np

if "/opt/trn_rl_repo" not in sys.path:
    sys.path.insert(0, "/opt/trn_rl_repo")

N_CORES = 8
N_ATOMS = 4_194_304
M_TOTAL = 4096
APM = 1024                   # atoms per molecule
M_LOC = M_TOTAL // N_CORES   # 512 molecules per core
N_LOC = N_ATOMS // N_CORES   # 524288 atoms per core
P = 128                      # partitions
G = M_LOC // P               # 4 groups of 128 molecules
INV_N = 1.0 / APM
NEWTON_ITERS = 3

_cache = {}


def _build_nc():
    import concourse.bacc as bacc
    import concourse.mybir as mybir
    from concourse import tile
    from concourse.tile import add_dep_helper
    from contextlib import ExitStack

    fp32 = mybir.dt.float32
    fp16 = mybir.dt.float16
    Act = mybir.ActivationFunctionType
    Alu = mybir.AluOpType

    nc = bacc.Bacc(
        "TRN2",
        target_bir_lowering=False,
        debug=False,
        enable_asserts=False,
        num_devices=N_CORES,
    )
    c1d = nc.dram_tensor("c1s", [3, N_LOC], fp16, kind="ExternalInput").ap()
    c2d = nc.dram_tensor("c2s", [3, N_LOC], fp16, kind="ExternalInput").ap()
    outd = nc.dram_tensor("out", [M_LOC], fp32, kind="ExternalOutput").ap()

    # [G, P, 3, APM]: per group, molecule on partition, comps separated
    c1t = c1d.rearrange("c (g p a) -> g p c a", g=G, p=P, a=APM)
    c2t = c2d.rearrange("c (g p a) -> g p c a", g=G, p=P, a=APM)

    with ExitStack() as ctx:
        tc = ctx.enter_context(tile.TileContext(nc))
        io_pool = ctx.enter_context(tc.tile_pool(name="io", bufs=3))
        scr_pool = ctx.enter_context(tc.tile_pool(name="scr", bufs=2))
        st_pool = ctx.enter_context(tc.tile_pool(name="st", bufs=1))
        eig_pool = ctx.enter_context(tc.tile_pool(name="eig", bufs=1))

        # stats layouts: column = stat_index * G + group (all fp32)
        stats_v = st_pool.tile([P, 9 * G], fp32, tag="stats_v")  # s2s1[i*3+j]
        stats_a = st_pool.tile([P, 5 * G], fp32, tag="stats_a")  # q1,q2,m1[j]
        stats_g = st_pool.tile([P, 3 * G], fp32, tag="stats_g")  # m2[i]

        ones = st_pool.tile([P, APM], fp16, tag="ones")
        nc.vector.memset(ones[:], 1.0)

        prev_load = None
        for g in range(G):
            t1 = io_pool.tile([P, 3 * APM], fp16, tag="t1")
            t2 = io_pool.tile([P, 3 * APM], fp16, tag="t2")
            t1v = t1[:].rearrange("p (c a) -> p c a", c=3)
            t2v = t2[:].rearrange("p (c a) -> p c a", c=3)
            d1 = nc.sync.dma_start(t1v, c1t[g])
            d2 = nc.sync.dma_start(t2v, c2t[g])
            if prev_load is not None:
                # serialize group loads so group 0 isn't diluted by
                # round-robin with later groups (pipeline ramp)
                add_dep_helper(d1.ins, prev_load.ins, sync=True,
                               reason="serialize group loads")
            prev_load = d2

            # --- 9 cross sums on VectorE (fused multiply+reduce via STT) ---
            for i in range(3):
                for j in range(3):
                    scr_v = scr_pool.tile([P, APM], fp16, tag="scr_v")
                    col = (i * 3 + j) * G + g
                    nc.vector.scalar_tensor_tensor(
                        scr_v[:], t2v[:, i, :], 1.0, t1v[:, j, :],
                        Alu.mult, Alu.mult,
                        accum_out=stats_v[:, col : col + 1],
                    )

            # --- first moments: m2_x, m2_y on VectorE vs ones ---
            for i in range(2):
                scr_m2 = scr_pool.tile([P, APM], fp16, tag="scr_m2")
                col = i * G + g
                nc.vector.scalar_tensor_tensor(
                    scr_m2[:], t2v[:, i, :], 1.0, ones[:],
                    Alu.mult, Alu.mult,
                    accum_out=stats_g[:, col : col + 1],
                )

            # --- squared norms (full row) + m1 (3) + m2_z on ScalarE ---
            scr_a1 = scr_pool.tile([P, 3 * APM], fp16, tag="scr_a1")
            nc.scalar.activation(
                scr_a1[:], t1[:], Act.Square,
                accum_out=stats_a[:, 0 * G + g : 0 * G + g + 1],
            )
            scr_a2 = scr_pool.tile([P, 3 * APM], fp16, tag="scr_a2")
            nc.scalar.activation(
                scr_a2[:], t2[:], Act.Square,
                accum_out=stats_a[:, 1 * G + g : 1 * G + g + 1],
            )
            for j in range(3):
                scr_m = scr_pool.tile([P, APM], fp16, tag="scr_m")
                col = (2 + j) * G + g
                nc.scalar.activation(
                    scr_m[:], t1v[:, j, :], Act.Copy,
                    accum_out=stats_a[:, col : col + 1],
                )
            scr_a3 = scr_pool.tile([P, APM], fp16, tag="scr_a3")
            nc.scalar.activation(
                scr_a3[:], t2v[:, 2, :], Act.Copy,
                accum_out=stats_g[:, 2 * G + g : 2 * G + g + 1],
            )

        # ================= [M] stage: quartic root, loss =================
        VE = nc.vector

        def ET(name, w=G):
            return eig_pool.tile([P, w], fp32, tag=name, name=name)

        # views
        s2s1_ijg = stats_v[:].rearrange("p (ij g) -> p ij g", g=G)      # [P,9,G]
        m1_jg = stats_a[:, 2 * G : 5 * G].rearrange("p (j g) -> p j g", j=3)
        m2_ig = stats_g[:].rearrange("p (i g) -> p i g", i=3)
        q1_g = stats_a[:, 0:G]
        q2_g = stats_a[:, G : 2 * G]

        # outer[i,j,g] = m2_i * m1_j
        outer = ET("outer", 9 * G)
        outer_v = outer[:].rearrange("p (i j g) -> p i j g", i=3, j=3)
        VE.tensor_tensor(
            outer_v,
            m2_ig.unsqueeze(2).broadcast_to([P, 3, 3, G]),
            m1_jg.unsqueeze(1).broadcast_to([P, 3, 3, G]),
            Alu.mult,
        )
        # Cp = s2s1 - outer/n   (= n * cov)
        Cp = ET("Cp", 9 * G)
        Cp_ijg = Cp[:].rearrange("p (ij g) -> p ij g", g=G)
        VE.scalar_tensor_tensor(
            Cp_ijg,
            outer[:].rearrange("p (ij g) -> p ij g", g=G),
            -INV_N,
            s2s1_ijg,
            Alu.mult,
            Alu.add,
        )

        # v1p = q1 - |m1|^2 / n ; v2p likewise
        def vterm(m_gj_view, q_view, nm):
            mm = ET("mm_" + nm, 3 * G)
            mm_v = mm[:].rearrange("p (g j) -> p g j", j=3)
            VE.tensor_tensor(mm_v, m_gj_view, m_gj_view, Alu.mult)
            sq = ET("sq_" + nm)
            VE.tensor_reduce(sq[:], mm_v, axis=mybir.AxisListType.X, op=Alu.add)
            vp = ET("vp_" + nm)
            VE.scalar_tensor_tensor(vp[:], sq[:], -INV_N, q_view, Alu.mult, Alu.add)
            return vp

        m1_gj = stats_a[:, 2 * G : 5 * G].rearrange("p (j g) -> p g j", j=3)
        m2_gi = stats_g[:].rearrange("p (i g) -> p g i", i=3)
        v1p = vterm(m1_gj, q1_g, "1")
        v2p = vterm(m2_gi, q2_g, "2")

        # e1 = |Cp|_F^2 per group
        Cp_gij = Cp[:].rearrange("p (ij g) -> p g ij", g=G)
        csq = ET("csq", 9 * G)
        csq_v = csq[:].rearrange("p (g ij) -> p g ij", g=G)
        VE.tensor_tensor(csq_v, Cp_gij, Cp_gij, Alu.mult)
        e1 = ET("e1")
        VE.tensor_reduce(e1[:], csq_v, axis=mybir.AxisListType.X, op=Alu.add)

        # A = Cp^T Cp, layout col = g*9 + i*3 + j
        Cp_kig = Cp[:].rearrange("p (k i g) -> p k i g", k=3, i=3)
        A = ET("A", 9 * G)
        A_w = A[:].rearrange("p (g i j) -> p i j g", g=G, i=3, j=3)
        Ak = ET("Ak", 9 * G)
        Ak_w = Ak[:].rearrange("p (g i j) -> p i j g", g=G, i=3, j=3)
        for k in range(3):
            rowk = Cp_kig[:, k, :, :]  # [P, 3, G] = Cp[k, i]
            dst = A_w if k == 0 else Ak_w
            VE.tensor_tensor(
                dst,
                rowk.unsqueeze(2).broadcast_to([P, 3, 3, G]),
                rowk.unsqueeze(1).broadcast_to([P, 3, 3, G]),
                Alu.mult,
            )
            if k > 0:
                VE.tensor_tensor(A[:], A[:], Ak[:], Alu.add)

        # trA2 = |A|_F^2 per group
        A_gij = A[:].rearrange("p (g ij) -> p g ij", g=G)
        asq = ET("asq", 9 * G)
        asq_v = asq[:].rearrange("p (g ij) -> p g ij", g=G)
        VE.tensor_tensor(asq_v, A_gij, A_gij, Alu.mult)
        trA2 = ET("trA2")
        VE.tensor_reduce(trA2[:], asq_v, axis=mybir.AxisListType.X, op=Alu.add)

        # det(Cp): P12[a,b] = Cp[1,a]*Cp[2,b]; cofactors; dot with row 0
        row1 = Cp_kig[:, 1, :, :]
        row2 = Cp_kig[:, 2, :, :]
        P12 = ET("P12", 9 * G)
        P12_v = P12[:].rearrange("p (a b g) -> p a b g", a=3, b=3)
        VE.tensor_tensor(
            P12_v,
            row1.unsqueeze(2).broadcast_to([P, 3, 3, G]),
            row2.unsqueeze(1).broadcast_to([P, 3, 3, G]),
            Alu.mult,
        )

        def p12s(a, b):
            c = (a * 3 + b) * G
            return P12[:, c : c + G]

        cof = ET("cof", 3 * G)  # layout (j, g)
        VE.tensor_tensor(cof[:, 0:G], p12s(1, 2), p12s(2, 1), Alu.subtract)
        VE.tensor_tensor(cof[:, G : 2 * G], p12s(2, 0), p12s(0, 2), Alu.subtract)
        VE.tensor_tensor(cof[:, 2 * G : 3 * G], p12s(0, 1), p12s(1, 0), Alu.subtract)
        dp = ET("dp", 3 * G)
        VE.tensor_tensor(dp[:], Cp[:, 0 : 3 * G], cof[:], Alu.mult)
        det = ET("det")
        VE.tensor_reduce(
            det[:],
            dp[:].rearrange("p (j g) -> p g j", j=3),
            axis=mybir.AxisListType.X,
            op=Alu.add,
        )

        # quartic coefficients
        e1sq = ET("e1sq")
        VE.tensor_tensor(e1sq[:], e1[:], e1[:], Alu.mult)
        e2h = ET("e2h")  # = e1^2 - trA2 = 2*e2
        VE.tensor_tensor(e2h[:], e1sq[:], trA2[:], Alu.subtract)
        VE.tensor_scalar(e2h[:], e2h[:], 0.0, None, Alu.max)
        sq2 = ET("sq2")  # sqrt(3*e2) = sqrt(1.5 * e2h)
        nc.scalar.activation(sq2[:], e2h[:], Act.Sqrt, scale=1.5)
        arg = ET("arg")  # e1 + 2*sq2
        VE.scalar_tensor_tensor(arg[:], sq2[:], 2.0, e1[:], Alu.mult, Alu.add)
        s = ET("s")      # Newton iterate, start = upper bound of largest root
        nc.scalar.activation(s[:], arg[:], Act.Sqrt)
        c0 = ET("c0")    # 2*trA2 - e1^2
        VE.scalar_tensor_tensor(c0[:], trA2[:], 2.0, e1sq[:], Alu.mult, Alu.subtract)
        e1x2 = ET("e1x2")
        VE.tensor_scalar(e1x2[:], e1[:], 2.0, None, Alu.mult)
        d8 = ET("d8")
        VE.tensor_scalar(d8[:], det[:], 8.0, None, Alu.mult)
        d2 = ET("d2")
        VE.tensor_scalar(d2[:], det[:], 2.0, None, Alu.mult)

        # Newton: s -= P(s)/P'(s);  P = s^4 - 2 e1 s^2 - 8 d s + c0,
        # P' = 4 (s^3 - e1 s - 2 d)
        for _ in range(NEWTON_ITERS):
            s2t = ET("s2t")
            VE.tensor_tensor(s2t[:], s[:], s[:], Alu.mult)
            t1t = ET("t1t")
            VE.tensor_tensor(t1t[:], s2t[:], e1x2[:], Alu.subtract)
            Pt = ET("Pt")
            VE.tensor_tensor(Pt[:], t1t[:], s2t[:], Alu.mult)
            ut = ET("ut")
            VE.tensor_tensor(ut[:], d8[:], s[:], Alu.mult)
            VE.tensor_tensor(Pt[:], Pt[:], ut[:], Alu.subtract)
            VE.tensor_tensor(Pt[:], Pt[:], c0[:], Alu.add)
            s3t = ET("s3t")
            VE.tensor_tensor(s3t[:], s2t[:], s[:], Alu.mult)
            wt = ET("wt")
            VE.tensor_tensor(wt[:], e1[:], s[:], Alu.mult)
            VE.tensor_tensor(wt[:], s3t[:], wt[:], Alu.subtract)
            VE.tensor_tensor(wt[:], wt[:], d2[:], Alu.subtract)
            VE.tensor_scalar(wt[:], wt[:], 1.0, None, Alu.max)
            winv = ET("winv")
            VE.reciprocal(winv[:], wt[:])
            dlt = ET("dlt")
            VE.scalar_tensor_tensor(dlt[:], Pt[:], 0.25, winv[:], Alu.mult, Alu.mult)
            VE.tensor_tensor(s[:], s[:], dlt[:], Alu.subtract)

        # loss = (v1p + v2p - 2*s) / (3n)
        vsum = ET("vsum")
        VE.tensor_tensor(vsum[:], v1p[:], v2p[:], Alu.add)
        num = ET("num")
        VE.scalar_tensor_tensor(num[:], s[:], -2.0, vsum[:], Alu.mult, Alu.add)
        loss = ET("loss")
        VE.tensor_scalar(loss[:], num[:], 1.0 / (3.0 * APM), None, Alu.mult)

        nc.sync.dma_start(outd.rearrange("(g p) -> p g", p=P), loss[:])

    nc.compile()
    return nc


def _get_nc():
    if "nc" not in _cache:
        _cache["nc"] = _build_nc()
    return _cache["nc"]


def _numpy_fallback(coords1, coords2, mol_ids, num_molecules):
    """Correct host implementation for unexpected input patterns."""
    M = int(num_molecules)
    c1 = np.asarray(coords1, dtype=np.float64)
    c2 = np.asarray(coords2, dtype=np.float64)
    ids = np.asarray(mol_ids)
    cnt = np.bincount(ids, minlength=M).astype(np.float64)
    s1 = np.zeros((M, 3)); s2 = np.zeros((M, 3))
    np.add.at(s1, ids, c1); np.add.at(s2, ids, c2)
    cnt_safe = np.maximum(cnt, 1.0)
    mu1 = s1 / cnt_safe[:, None]; mu2 = s2 / cnt_safe[:, None]
    d1 = c1 - mu1[ids]; d2 = c2 - mu2[ids]
    cov = np.zeros((M, 3, 3))
    np.add.at(cov, ids, d2[:, :, None] * d1[:, None, :])
    cov /= cnt[:, None, None]
    sig = np.linalg.svd(cov, compute_uv=False)
    detc = np.linalg.det(cov)
    sig[:, -1] *= np.where(detc < 0, -1.0, 1.0)
    v1 = np.zeros(M); v2 = np.zeros(M)
    np.add.at(v1, ids, (d1 * d1).sum(1)); np.add.at(v2, ids, (d2 * d2).sum(1))
    v1 /= 3 * cnt; v2 /= 3 * cnt
    return (v1 + v2 - 2.0 * sig.mean(1)).astype(np.float32)


_EXPECTED_IDS = None


def _is_standard_pattern(coords1, coords2, mol_ids, num_molecules):
    global _EXPECTED_IDS
    if int(num_molecules) != M_TOTAL:
        return False
    if coords1.shape != (N_ATOMS, 3) or coords2.shape != (N_ATOMS, 3):
        return False
    if _EXPECTED_IDS is None:
        _EXPECTED_IDS = np.repeat(
            np.arange(M_TOTAL, dtype=np.int64), N_ATOMS // M_TOTAL
        )
    return bool(np.array_equal(np.asarray(mol_ids, dtype=np.int64), _EXPECTED_IDS))


def kernel(coords1, coords2, mol_ids, num_molecules, _trace=False):
    coords1 = np.asarray(coords1)
    coords2 = np.asarray(coords2)
    if not _is_standard_pattern(coords1, coords2, mol_ids, num_molecules):
        return _numpy_fallback(coords1, coords2, mol_ids, num_molecules)

    from concourse.bass_utils import run_bass_kernel_spmd

    nc = _get_nc()
    # host pre-pass: per-core component-separated fp16 [3, N_LOC]
    h1 = coords1.astype(np.float16)
    h2 = coords2.astype(np.float16)
    in_maps = []
    for i in range(N_CORES):
        sl = slice(i * N_LOC, (i + 1) * N_LOC)
        in_maps.append({
            "c1s": np.ascontiguousarray(h1[sl].T),
            "c2s": np.ascontiguousarray(h2[sl].T),
        })
    res = run_bass_kernel_spmd(nc, in_maps, list(range(N_CORES)), trace=_trace)
    out = np.concatenate([res.results[i]["out"] for i in range(N_CORES)])
    if _trace:
        _cache["last_results"] = res
    return out.astype(np.float32, copy=False)


# revision 15
# speedup vs baseline: 1.1654x; 1.1654x over previous
"""PolymerDistance loss kernel for 8 Trainium2 NeuronCores.

Math (per molecule m of exactly 1024 atoms, Kabsch-style loss):
  loss[m] = var1 + var2 - 2*mean(sigma)  where sigma are singular values of the
  3x3 cross covariance of centered coords, with the smallest one sign-corrected
  by sign(det(cov)).

Device strategy:
  - Shard 4096 molecules evenly: 512 molecules (524288 atoms) per core. No
    collectives needed (molecules never straddle a shard).
  - Host pre-pass converts coords to component-separated fp16 ([3, N] per
    tensor). Input rounding error washes out over the 1024-atom averages
    (measured: identical L2 error to an all-f32 device path), while halving
    HBM traffic and enabling the DVE 2x perf mode (16-bit, unit-stride).
    All accumulation stays fp32 on device.
  - Per core, 4 groups of 128 molecules; one molecule per SBUF partition,
    free dim = [3 components x 1024 atoms] contiguous per component.
  - Sufficient statistics per molecule (18 f32 sums): 9 crosses
    sum(x2_i*x1_j), 6 per-component square sums, 3+3 first moments.
    Fused multiply+reduce via scalar_tensor_tensor(accum_out) on VectorE
    (moments multiply against a constant ones tile); ScalarE takes the
    squares (activation Square + accum_out) plus one moment to balance.
  - Tiny [M] stage stays on device: sigma1+sigma2+sign(det)*sigma3 is the
    largest root of  s^4 - 2*e1*s^2 - 8*det*s + (e1^2 - 4*e2)  where
    e1 = tr(C^T C) = |C|_F^2, e2 = (e1^2 - |C^T C|_F^2)/2, for C = n*cov.
    Solved with a tight initializer + 3 Newton steps, all elementwise on
    [128, 4] tiles. No trig, single Sqrt activation table.
"""

import sys

import numpy as np

if "/opt/trn_rl_repo" not in sys.path:
    sys.path.insert(0, "/opt/trn_rl_repo")

N_CORES = 8
N_ATOMS = 4_194_304
M_TOTAL = 4096
APM = 1024                   # atoms per molecule
M_LOC = M_TOTAL // N_CORES   # 512 molecules per core
N_LOC = N_ATOMS // N_CORES   # 524288 atoms per core
P = 128                      # partitions
G = M_LOC // P               # 4 groups of 128 molecules
INV_N = 1.0 / APM
NEWTON_ITERS = 2

_cache = {}


def _build_nc():
    import concourse.bacc as bacc
    import concourse.mybir as mybir
    from concourse import tile
    from concourse.tile import add_dep_helper
    from contextlib import ExitStack

    fp32 = mybir.dt.float32
    fp16 = mybir.dt.float16
    Act = mybir.ActivationFunctionType
    Alu = mybir.AluOpType

    nc = bacc.Bacc(
        "TRN2",
        target_bir_lowering=False,
        debug=False,
        enable_asserts=False,
        num_devices=N_CORES,
    )
    c1d = nc.dram_tensor("c1s", [3, N_LOC], fp16, kind="ExternalInput").ap()
    c2d = nc.dram_tensor("c2s", [3, N_LOC], fp16, kind="ExternalInput").ap()
    outd = nc.dram_tensor("out", [M_LOC], fp32, kind="ExternalOutput").ap()

    # [G, P, 3, APM]: per group, molecule on partition, comps separated
    c1t = c1d.rearrange("c (g p a) -> g p c a", g=G, p=P, a=APM)
    c2t = c2d.rearrange("c (g p a) -> g p c a", g=G, p=P, a=APM)

    with ExitStack() as ctx:
        tc = ctx.enter_context(tile.TileContext(nc))
        io_pool = ctx.enter_context(tc.tile_pool(name="io", bufs=3))
        scr_pool = ctx.enter_context(tc.tile_pool(name="scr", bufs=2))
        st_pool = ctx.enter_context(tc.tile_pool(name="st", bufs=1))
        eig_pool = ctx.enter_context(tc.tile_pool(name="eig", bufs=1))

        # stats layouts: column = stat_index * G + group (all fp32)
        stats_v = st_pool.tile([P, 9 * G], fp32, tag="stats_v")  # s2s1[i*3+j]
        stats_a = st_pool.tile([P, 5 * G], fp32, tag="stats_a")  # q1,q2,m1[j]
        stats_g = st_pool.tile([P, 3 * G], fp32, tag="stats_g")  # m2[i]

        ones = st_pool.tile([P, APM], fp16, tag="ones")
        nc.vector.memset(ones[:], 1.0)

        prev_load = None
        for g in range(G):
            t1 = io_pool.tile([P, 3 * APM], fp16, tag="t1")
            t2 = io_pool.tile([P, 3 * APM], fp16, tag="t2")
            t1v = t1[:].rearrange("p (c a) -> p c a", c=3)
            t2v = t2[:].rearrange("p (c a) -> p c a", c=3)
            d1 = nc.sync.dma_start(t1v, c1t[g])
            if prev_load is not None:
                # serialize loads so earlier tiles aren't diluted by
                # packet round-robin with later ones (pipeline ramp)
                add_dep_helper(d1.ins, prev_load.ins, sync=True,
                               reason="serialize loads")
            d2 = nc.sync.dma_start(t2v, c2t[g])
            add_dep_helper(d2.ins, d1.ins, sync=True,
                           reason="serialize loads")
            prev_load = d2

            # --- 9 cross sums on VectorE (fused multiply+reduce via STT) ---
            for i in range(3):
                for j in range(3):
                    scr_v = scr_pool.tile([P, APM], fp16, tag="scr_v")
                    col = (i * 3 + j) * G + g
                    nc.vector.scalar_tensor_tensor(
                        scr_v[:], t2v[:, i, :], 1.0, t1v[:, j, :],
                        Alu.mult, Alu.mult,
                        accum_out=stats_v[:, col : col + 1],
                    )

            # --- first moments: m2_x, m2_y on ScalarE too (V is critical) ---
            for i in range(2):
                scr_m2 = scr_pool.tile([P, APM], fp16, tag="scr_m2")
                col = i * G + g
                nc.scalar.activation(
                    scr_m2[:], t2v[:, i, :], Act.Copy,
                    accum_out=stats_g[:, col : col + 1],
                )

            # --- squared norms (full row) + m1 (3) + m2_z on ScalarE ---
            scr_a1 = scr_pool.tile([P, 3 * APM], fp16, tag="scr_a1")
            nc.scalar.activation(
                scr_a1[:], t1[:], Act.Square,
                accum_out=stats_a[:, 0 * G + g : 0 * G + g + 1],
            )
            scr_a2 = scr_pool.tile([P, 3 * APM], fp16, tag="scr_a2")
            nc.scalar.activation(
                scr_a2[:], t2[:], Act.Square,
                accum_out=stats_a[:, 1 * G + g : 1 * G + g + 1],
            )
            for j in range(3):
                scr_m = scr_pool.tile([P, APM], fp16, tag="scr_m")
                col = (2 + j) * G + g
                nc.scalar.activation(
                    scr_m[:], t1v[:, j, :], Act.Copy,
                    accum_out=stats_a[:, col : col + 1],
                )
            scr_a3 = scr_pool.tile([P, APM], fp16, tag="scr_a3")
            nc.scalar.activation(
                scr_a3[:], t2v[:, 2, :], Act.Copy,
                accum_out=stats_g[:, 2 * G + g : 2 * G + g + 1],
            )

        # ================= [M] stage: quartic root, loss =================
        VE = nc.vector

        def ET(name, w=G):
            return eig_pool.tile([P, w], fp32, tag=name, name=name)

        # views
        s2s1_ijg = stats_v[:].rearrange("p (ij g) -> p ij g", g=G)      # [P,9,G]
        m1_jg = stats_a[:, 2 * G : 5 * G].rearrange("p (j g) -> p j g", j=3)
        m2_ig = stats_g[:].rearrange("p (i g) -> p i g", i=3)
        q1_g = stats_a[:, 0:G]
        q2_g = stats_a[:, G : 2 * G]

        # outer[i,j,g] = m2_i * m1_j
        outer = ET("outer", 9 * G)
        outer_v = outer[:].rearrange("p (i j g) -> p i j g", i=3, j=3)
        VE.tensor_tensor(
            outer_v,
            m2_ig.unsqueeze(2).broadcast_to([P, 3, 3, G]),
            m1_jg.unsqueeze(1).broadcast_to([P, 3, 3, G]),
            Alu.mult,
        )
        # Cp = s2s1 - outer/n   (= n * cov)
        Cp = ET("Cp", 9 * G)
        Cp_ijg = Cp[:].rearrange("p (ij g) -> p ij g", g=G)
        VE.scalar_tensor_tensor(
            Cp_ijg,
            outer[:].rearrange("p (ij g) -> p ij g", g=G),
            -INV_N,
            s2s1_ijg,
            Alu.mult,
            Alu.add,
        )

        # v1p = q1 - |m1|^2 / n ; v2p likewise
        def vterm(m_gj_view, q_view, nm):
            mm = ET("mm_" + nm, 3 * G)
            mm_v = mm[:].rearrange("p (g j) -> p g j", j=3)
            VE.tensor_tensor(mm_v, m_gj_view, m_gj_view, Alu.mult)
            sq = ET("sq_" + nm)
            VE.tensor_reduce(sq[:], mm_v, axis=mybir.AxisListType.X, op=Alu.add)
            vp = ET("vp_" + nm)
            VE.scalar_tensor_tensor(vp[:], sq[:], -INV_N, q_view, Alu.mult, Alu.add)
            return vp

        m1_gj = stats_a[:, 2 * G : 5 * G].rearrange("p (j g) -> p g j", j=3)
        m2_gi = stats_g[:].rearrange("p (i g) -> p g i", i=3)
        v1p = vterm(m1_gj, q1_g, "1")
        v2p = vterm(m2_gi, q2_g, "2")

        # e1 = |Cp|_F^2 per group
        Cp_gij = Cp[:].rearrange("p (ij g) -> p g ij", g=G)
        csq = ET("csq", 9 * G)
        csq_v = csq[:].rearrange("p (g ij) -> p g ij", g=G)
        VE.tensor_tensor(csq_v, Cp_gij, Cp_gij, Alu.mult)
        e1 = ET("e1")
        VE.tensor_reduce(e1[:], csq_v, axis=mybir.AxisListType.X, op=Alu.add)

        # A = Cp^T Cp, layout col = g*9 + i*3 + j
        Cp_kig = Cp[:].rearrange("p (k i g) -> p k i g", k=3, i=3)
        A = ET("A", 9 * G)
        A_w = A[:].rearrange("p (g i j) -> p i j g", g=G, i=3, j=3)
        Ak = ET("Ak", 9 * G)
        Ak_w = Ak[:].rearrange("p (g i j) -> p i j g", g=G, i=3, j=3)
        for k in range(3):
            rowk = Cp_kig[:, k, :, :]  # [P, 3, G] = Cp[k, i]
            dst = A_w if k == 0 else Ak_w
            VE.tensor_tensor(
                dst,
                rowk.unsqueeze(2).broadcast_to([P, 3, 3, G]),
                rowk.unsqueeze(1).broadcast_to([P, 3, 3, G]),
                Alu.mult,
            )
            if k > 0:
                VE.tensor_tensor(A[:], A[:], Ak[:], Alu.add)

        # trA2 = |A|_F^2 per group
        A_gij = A[:].rearrange("p (g ij) -> p g ij", g=G)
        asq = ET("asq", 9 * G)
        asq_v = asq[:].rearrange("p (g ij) -> p g ij", g=G)
        VE.tensor_tensor(asq_v, A_gij, A_gij, Alu.mult)
        trA2 = ET("trA2")
        VE.tensor_reduce(trA2[:], asq_v, axis=mybir.AxisListType.X, op=Alu.add)

        # det(Cp): P12[a,b] = Cp[1,a]*Cp[2,b]; cofactors; dot with row 0
        row1 = Cp_kig[:, 1, :, :]
        row2 = Cp_kig[:, 2, :, :]
        P12 = ET("P12", 9 * G)
        P12_v = P12[:].rearrange("p (a b g) -> p a b g", a=3, b=3)
        VE.tensor_tensor(
            P12_v,
            row1.unsqueeze(2).broadcast_to([P, 3, 3, G]),
            row2.unsqueeze(1).broadcast_to([P, 3, 3, G]),
            Alu.mult,
        )

        def p12s(a, b):
            c = (a * 3 + b) * G
            return P12[:, c : c + G]

        cof = ET("cof", 3 * G)  # layout (j, g)
        VE.tensor_tensor(cof[:, 0:G], p12s(1, 2), p12s(2, 1), Alu.subtract)
        VE.tensor_tensor(cof[:, G : 2 * G], p12s(2, 0), p12s(0, 2), Alu.subtract)
        VE.tensor_tensor(cof[:, 2 * G : 3 * G], p12s(0, 1), p12s(1, 0), Alu.subtract)
        dp = ET("dp", 3 * G)
        VE.tensor_tensor(dp[:], Cp[:, 0 : 3 * G], cof[:], Alu.mult)
        det = ET("det")
        VE.tensor_reduce(
            det[:],
            dp[:].rearrange("p (j g) -> p g j", j=3),
            axis=mybir.AxisListType.X,
            op=Alu.add,
        )

        # quartic coefficients
        e1sq = ET("e1sq")
        VE.tensor_tensor(e1sq[:], e1[:], e1[:], Alu.mult)
        e2h = ET("e2h")  # = e1^2 - trA2 = 2*e2
        VE.tensor_tensor(e2h[:], e1sq[:], trA2[:], Alu.subtract)
        VE.tensor_scalar(e2h[:], e2h[:], 0.0, None, Alu.max)
        sq2 = ET("sq2")  # sqrt(3*e2) = sqrt(1.5 * e2h)
        nc.scalar.activation(sq2[:], e2h[:], Act.Sqrt, scale=1.5)
        arg = ET("arg")  # e1 + 2*sq2
        VE.scalar_tensor_tensor(arg[:], sq2[:], 2.0, e1[:], Alu.mult, Alu.add)
        s = ET("s")      # Newton iterate, start = upper bound of largest root
        nc.scalar.activation(s[:], arg[:], Act.Sqrt)
        c0 = ET("c0")    # 2*trA2 - e1^2
        VE.scalar_tensor_tensor(c0[:], trA2[:], 2.0, e1sq[:], Alu.mult, Alu.subtract)
        e1x2 = ET("e1x2")
        VE.tensor_scalar(e1x2[:], e1[:], 2.0, None, Alu.mult)
        d8 = ET("d8")
        VE.tensor_scalar(d8[:], det[:], 8.0, None, Alu.mult)
        d2 = ET("d2")
        VE.tensor_scalar(d2[:], det[:], 2.0, None, Alu.mult)

        # Newton: s -= P(s)/P'(s);  P = s^4 - 2 e1 s^2 - 8 d s + c0,
        # P' = 4 (s^3 - e1 s - 2 d)
        for _ in range(NEWTON_ITERS):
            s2t = ET("s2t")
            VE.tensor_tensor(s2t[:], s[:], s[:], Alu.mult)
            t1t = ET("t1t")
            VE.tensor_tensor(t1t[:], s2t[:], e1x2[:], Alu.subtract)
            Pt = ET("Pt")
            VE.tensor_tensor(Pt[:], t1t[:], s2t[:], Alu.mult)
            ut = ET("ut")
            VE.tensor_tensor(ut[:], d8[:], s[:], Alu.mult)
            VE.tensor_tensor(Pt[:], Pt[:], ut[:], Alu.subtract)
            VE.tensor_tensor(Pt[:], Pt[:], c0[:], Alu.add)
            s3t = ET("s3t")
            VE.tensor_tensor(s3t[:], s2t[:], s[:], Alu.mult)
            wt = ET("wt")
            VE.tensor_tensor(wt[:], e1[:], s[:], Alu.mult)
            VE.tensor_tensor(wt[:], s3t[:], wt[:], Alu.subtract)
            VE.tensor_tensor(wt[:], wt[:], d2[:], Alu.subtract)
            VE.tensor_scalar(wt[:], wt[:], 1.0, None, Alu.max)
            winv = ET("winv")
            VE.reciprocal(winv[:], wt[:])
            dlt = ET("dlt")
            VE.scalar_tensor_tensor(dlt[:], Pt[:], 0.25, winv[:], Alu.mult, Alu.mult)
            VE.tensor_tensor(s[:], s[:], dlt[:], Alu.subtract)

        # loss = (v1p + v2p - 2*s) / (3n)
        vsum = ET("vsum")
        VE.tensor_tensor(vsum[:], v1p[:], v2p[:], Alu.add)
        num = ET("num")
        VE.scalar_tensor_tensor(num[:], s[:], -2.0, vsum[:], Alu.mult, Alu.add)
        loss = ET("loss")
        VE.tensor_scalar(loss[:], num[:], 1.0 / (3.0 * APM), None, Alu.mult)

        nc.sync.dma_start(outd.rearrange("(g p) -> p g", p=P), loss[:])

    nc.compile()
    return nc


def _get_nc():
    if "nc" not in _cache:
        _cache["nc"] = _build_nc()
    return _cache["nc"]


def _numpy_fallback(coords1, coords2, mol_ids, num_molecules):
    """Correct host implementation for unexpected input patterns."""
    M = int(num_molecules)
    c1 = np.asarray(coords1, dtype=np.float64)
    c2 = np.asarray(coords2, dtype=np.float64)
    ids = np.asarray(mol_ids)
    cnt = np.bincount(ids, minlength=M).astype(np.float64)
    s1 = np.zeros((M, 3)); s2 = np.zeros((M, 3))
    np.add.at(s1, ids, c1); np.add.at(s2, ids, c2)
    cnt_safe = np.maximum(cnt, 1.0)
    mu1 = s1 / cnt_safe[:, None]; mu2 = s2 / cnt_safe[:, None]
    d1 = c1 - mu1[ids]; d2 = c2 - mu2[ids]
    cov = np.zeros((M, 3, 3))
    np.add.at(cov, ids, d2[:, :, None] * d1[:, None, :])
    cov /= cnt[:, None, None]
    sig = np.linalg.svd(cov, compute_uv=False)
    detc = np.linalg.det(cov)
    sig[:, -1] *= np.where(detc < 0, -1.0, 1.0)
    v1 = np.zeros(M); v2 = np.zeros(M)
    np.add.at(v1, ids, (d1 * d1).sum(1)); np.add.at(v2, ids, (d2 * d2).sum(1))
    v1 /= 3 * cnt; v2 /= 3 * cnt
    return (v1 + v2 - 2.0 * sig.mean(1)).astype(np.float32)


_EXPECTED_IDS = None


def _is_standard_pattern(coords1, coords2, mol_ids, num_molecules):
    global _EXPECTED_IDS
    if int(num_molecules) != M_TOTAL:
        return False
    if coords1.shape != (N_ATOMS, 3) or coords2.shape != (N_ATOMS, 3):
        return False
    if _EXPECTED_IDS is None:
        _EXPECTED_IDS = np.repeat(
            np.arange(M_TOTAL, dtype=np.int64), N_ATOMS // M_TOTAL
        )
    return bool(np.array_equal(np.asarray(mol_ids, dtype=np.int64), _EXPECTED_IDS))


def kernel(coords1, coords2, mol_ids, num_molecules, _trace=False):
    coords1 = np.asarray(coords1)
    coords2 = np.asarray(coords2)
    if not _is_standard_pattern(coords1, coords2, mol_ids, num_molecules):
        return _numpy_fallback(coords1, coords2, mol_ids, num_molecules)

    from concourse.bass_utils import run_bass_kernel_spmd

    nc = _get_nc()
    # host pre-pass: per-core component-separated fp16 [3, N_LOC]
    h1 = coords1.astype(np.float16)
    h2 = coords2.astype(np.float16)
    in_maps = []
    for i in range(N_CORES):
        sl = slice(i * N_LOC, (i + 1) * N_LOC)
        in_maps.append({
            "c1s": np.ascontiguousarray(h1[sl].T),
            "c2s": np.ascontiguousarray(h2[sl].T),
        })
    res = run_bass_kernel_spmd(nc, in_maps, list(range(N_CORES)), trace=_trace)
    out = np.concatenate([res.results[i]["out"] for i in range(N_CORES)])
    if _trace:
        _cache["last_results"] = res
    return out.astype(np.float32, copy=False)
